# revision 1
# baseline (speedup 1.0000x reference)
"""Nystrom attention Trainium2 kernel.

Full-input contract: kernel(Q, K, V) with shapes [4, 16, 4096, 64] fp32,
returns X [4, 16, 4096, 64] fp32.  The 64 (batch, head) pairs are sharded
8-per-core across 8 NeuronCores; each core runs an identical program over
its 8 pairs (SPMD), no cross-core communication.

Per-pair math (S=4096, m=64 landmarks, d=64, seg=64):
  Qlm = seg-mean(Q) * s^2 (s^2 = 1/8 folded into pooling), Klm likewise
  L1^T = Klm_s2 @ Q^T            -> E1^T = exp(L1^T)          [m, S]
  L2   = Qlm_s2 @ Klm_raw^T      -> k2 = rownorm(exp(L2))     [m, m]
  L3^T = K @ Qlm_s2^T            -> E3^T = exp(L3^T)          [S, m]
  CVa  = E3^T.T @ [V | 1]        -> CV = rownorm via last col [m, d]
  B    = newton_schulz_inv(k2, 6 iters)
  M2'  = [B @ CV | 1]
  X'   = E1^T.T @ M2'            -> X = X'[:, :64] / X'[:, 64] [S, d]

Q^T/K^T are built by PE transpose-mode into PSUM, with the PSUM->SBUF
copies split across ScalarE (Q^T) and VectorE (K^T) to balance engines.
"""

import math

import numpy as np

import concourse.bass as bass
import concourse.tile as tile
from concourse import bacc, mybir

F32 = mybir.dt.float32
MMDT = mybir.dt.bfloat16

B, H, S, D = 4, 16, 4096, 64
M = 64            # landmarks
SEG = S // M      # 64
NT = S // 128     # 32 s-tiles per pair
N_CORES = 8
PAIRS = (B * H) // N_CORES  # 8 pairs per core
SCALE2 = 1.0 / math.sqrt(D)  # s^2 folded once into pooling weights
POOLW = SCALE2 / SEG

Exp = mybir.ActivationFunctionType.Exp
Alu = mybir.AluOpType
AX = mybir.AxisListType


def _consts():
    i128 = np.eye(128, dtype=np.float32)
    i64q = 3.25 * np.eye(64, dtype=np.float32)
    poolc = np.zeros((128, 2), dtype=np.float32)
    poolc[:64, 0] = POOLW
    poolc[64:, 1] = POOLW
    ones64 = np.ones((64, 1), dtype=np.float32)
    ones1x64 = np.ones((1, 64), dtype=np.float32)
    return i128, i64q, poolc, ones64, ones1x64


def build_body(tc, ctx, q_d, k_d, v_d, x_d, n_pairs):
    """Emit the per-core program. q_d/k_d/v_d/x_d: DRAM APs [n_pairs, S, D]."""
    nc = tc.nc
    i128_np, i64q_np, poolc_np, ones64_np, ones1x64_np = _consts()
    np_mm = mybir.dt.np(MMDT)

    i128_dram = nc.inline_tensor(i128_np.astype(np_mm), name="i128c")
    i64q_dram = nc.inline_tensor(i64q_np.astype(np_mm), name="i64qc")
    poolc_dram = nc.inline_tensor(poolc_np.astype(np_mm), name="poolcc")
    ones64_dram = nc.inline_tensor(ones64_np.astype(np_mm), name="ones64c")
    ones1x64_dram = nc.inline_tensor(ones1x64_np.astype(np_mm), name="ones1x64c")

    cpool = ctx.enter_context(tc.tile_pool(name="consts", bufs=1))
    inpool = ctx.enter_context(tc.tile_pool(name="inputs", bufs=3))
    tpool = ctx.enter_context(tc.tile_pool(name="trans", bufs=3))
    epool = ctx.enter_context(tc.tile_pool(name="exps", bufs=3))
    opool = ctx.enter_context(tc.tile_pool(name="outs", bufs=3))
    spool = ctx.enter_context(tc.tile_pool(name="smalls", bufs=3))
    ps_big = ctx.enter_context(tc.tile_pool(name="ps_big", bufs=3, space="PSUM"))
    ps_acc = ctx.enter_context(tc.tile_pool(name="ps_acc", bufs=1, space="PSUM"))
    ps_sm = ctx.enter_context(tc.tile_pool(name="ps_sm", bufs=3, space="PSUM"))

    i128 = cpool.tile([128, 128], MMDT)
    nc.sync.dma_start(out=i128[:], in_=i128_dram[:])
    i64 = i128[:64, :64]
    i64q = cpool.tile([64, 64], MMDT)
    nc.sync.dma_start(out=i64q[:], in_=i64q_dram[:])
    poolc = cpool.tile([128, 2], MMDT)
    nc.sync.dma_start(out=poolc[:], in_=poolc_dram[:])
    ones64 = cpool.tile([64, 1], MMDT)
    nc.sync.dma_start(out=ones64[:], in_=ones64_dram[:])
    ones1x64 = cpool.tile([1, 64], MMDT)
    nc.sync.dma_start(out=ones1x64[:], in_=ones1x64_dram[:])

    ldma = nc.sync if MMDT == F32 else nc.gpsimd  # SWDGE casts during DMA

    for p in range(n_pairs):
        qsrc = q_d[p].rearrange("(t p) d -> p t d", p=128)
        ksrc = k_d[p].rearrange("(t p) d -> p t d", p=128)
        vsrc = v_d[p].rearrange("(t p) d -> p t d", p=128)

        qn = inpool.tile([128, NT, 64], MMDT, tag="qn")
        ldma.dma_start(out=qn[:], in_=qsrc)
        kn = inpool.tile([128, NT, 64], MMDT, tag="kn")
        ldma.dma_start(out=kn[:], in_=ksrc)
        vn = inpool.tile([128, NT, 65], MMDT, tag="vn")
        ldma.dma_start(out=vn[:, :, 0:64], in_=vsrc)
        nc.vector.memset(vn[:, :, 64:65], 1.0)

        # ---- transposes: Q^T, K^T in SBUF [64, S] ----
        qt = tpool.tile([64, S], MMDT, tag="qt")
        kt = tpool.tile([64, S], MMDT, tag="kt")
        for src, dst, ceng in ((qn, qt, nc.scalar), (kn, kt, nc.vector)):
            for g in range(8):
                tp = ps_big.tile([64, 512], MMDT, tag="big")
                for j in range(4):
                    nc.tensor.transpose(
                        tp[:, 128 * j:128 * (j + 1)], src[:, 4 * g + j, :], i128[:]
                    )
                if ceng is nc.scalar:
                    nc.scalar.copy(dst[:, 512 * g:512 * (g + 1)], tp[:])
                else:
                    nc.vector.tensor_copy(dst[:, 512 * g:512 * (g + 1)], tp[:])

        # ---- landmark pooling (scale^2 folded): doubled [128, 64] stacks ----
        qlm_ps = ps_sm.tile([64, 64], F32, tag="sm")
        klm_ps = ps_sm.tile([64, 64], F32, tag="sm")
        for t in range(NT):
            nc.tensor.matmul(qlm_ps[:, 2 * t:2 * t + 2], qn[:, t, :], poolc[:])
            nc.tensor.matmul(klm_ps[:, 2 * t:2 * t + 2], kn[:, t, :], poolc[:])
        qlmT2 = spool.tile([64, 64], MMDT, tag="qlmT2")
        nc.vector.tensor_copy(qlmT2[:], qlm_ps[:])
        klmT2 = spool.tile([64, 64], MMDT, tag="klmT2")
        nc.vector.tensor_copy(klmT2[:], klm_ps[:])
        klmrT = spool.tile([64, 64], MMDT, tag="klmrT")
        nc.vector.tensor_scalar_mul(klmrT[:], klm_ps[:], 1.0 / SCALE2)

        # ---- kernel_2 = rownorm(exp(Qlm_s2 @ Klm_raw^T)) ----
        l2_ps = ps_sm.tile([64, 64], F32, tag="sm")
        nc.tensor.matmul(l2_ps[:], qlmT2[:], klmrT[:])
        e2 = spool.tile([64, 64], F32, tag="e2")
        d2 = spool.tile([64, 1], F32, tag="d2")
        nc.scalar.activation(e2[:], l2_ps[:], Exp, accum_out=d2[:])
        d2i = spool.tile([64, 1], F32, tag="d2i")
        nc.vector.reciprocal(d2i[:], d2[:])
        k2 = spool.tile([64, 64], MMDT, tag="k2")
        nc.vector.tensor_scalar_mul(k2[:], e2[:], d2i[:])
        k2t_ps = ps_sm.tile([64, 64], MMDT, tag="sm")
        nc.tensor.transpose(k2t_ps[:], k2[:], i64)
        k2t = spool.tile([64, 64], MMDT, tag="k2t")
        nc.vector.tensor_copy(k2t[:], k2t_ps[:])

        # ---- Newton-Schulz init: scale = 1/max(colsum) (rowsums are 1) ----
        c_ps = ps_sm.tile([64, 1], F32, tag="sm")
        nc.tensor.matmul(c_ps[:], k2[:], ones64[:])
        c_sb = spool.tile([64, 1], MMDT, tag="c_sb")
        nc.vector.tensor_copy(c_sb[:], c_ps[:])
        ct_ps = ps_sm.tile([1, 64], MMDT, tag="sm")
        nc.tensor.transpose(ct_ps[:], c_sb[:], i64)
        mx = spool.tile([1, 1], F32, tag="mx")
        nc.vector.tensor_reduce(mx[:], ct_ps[:], axis=AX.X, op=Alu.max)
        sci = spool.tile([1, 1], F32, tag="sci")
        nc.vector.reciprocal(sci[:], mx[:])
        sci_mm = spool.tile([1, 1], MMDT, tag="sci_mm")
        nc.vector.tensor_copy(sci_mm[:], sci[:])
        bc_ps = ps_sm.tile([64, 1], F32, tag="sm")
        nc.tensor.matmul(bc_ps[:], ones1x64[:], sci_mm[:])
        scb = spool.tile([64, 1], F32, tag="scb")
        nc.vector.tensor_copy(scb[:], bc_ps[:])

        vc = spool.tile([64, 64], MMDT, tag="vc0")
        nc.vector.tensor_scalar_mul(vc[:], k2t_ps[:], scb[:])
        vct = spool.tile([64, 64], MMDT, tag="vct0")
        nc.vector.tensor_scalar_mul(vct[:], k2[:], scb[:])

        # ---- 6 Newton-Schulz iterations (3rd order, 0.25 folded via i64q) ----
        for i in range(6):
            a_ps = ps_sm.tile([64, 64], F32, tag="sm")
            nc.tensor.matmul(a_ps[:], k2t[:], vc[:])
            at_ps = ps_sm.tile([64, 64], F32, tag="sm")
            nc.tensor.matmul(at_ps[:], vc[:], k2t[:])
            at_sb = spool.tile([64, 64], MMDT, tag="at_sb")
            nc.scalar.copy(at_sb[:], at_ps[:])
            b_sb = spool.tile([64, 64], MMDT, tag="b_sb")
            nc.vector.scalar_tensor_tensor(
                b_sb[:], i64, 7.0, a_ps[:], op0=Alu.mult, op1=Alu.subtract
            )
            cc_ps = ps_sm.tile([64, 64], F32, tag="sm")
            nc.tensor.matmul(cc_ps[:], at_sb[:], b_sb[:])
            d_sb = spool.tile([64, 64], MMDT, tag="d_sb")
            nc.vector.scalar_tensor_tensor(
                d_sb[:], i64, 15.0, cc_ps[:], op0=Alu.mult, op1=Alu.subtract
            )
            f_ps = ps_sm.tile([64, 64], F32, tag="sm")
            nc.tensor.matmul(f_ps[:], at_sb[:], d_sb[:])
            # g' = 3.25*I - 0.25*F  (= 0.25 * (13I - F))
            g_sb = spool.tile([64, 64], MMDT, tag="g_sb")
            nc.vector.scalar_tensor_tensor(
                g_sb[:], f_ps[:], -0.25, i64q[:], op0=Alu.mult, op1=Alu.add
            )
            vn_ps = ps_sm.tile([64, 64], F32, tag="sm")
            nc.tensor.matmul(vn_ps[:], vct[:], g_sb[:])
            vnt_ps = ps_sm.tile([64, 64], F32, tag="sm")
            nc.tensor.matmul(vnt_ps[:], g_sb[:], vct[:])
            vc = spool.tile([64, 64], MMDT, tag="vc", name=f"vc_{p}_{i}")
            nc.scalar.copy(vc[:], vn_ps[:])
            vct = spool.tile([64, 64], MMDT, tag="vct", name=f"vct_{p}_{i}")
            nc.vector.tensor_copy(vct[:], vnt_ps[:])

        # ---- kernel_3: E3^T [S, m] tiles, then CVa = E3^T.T @ [V|1] ----
        e3t = epool.tile([128, NT * 64], MMDT, tag="e3t")
        for g in range(4):
            l3_ps = ps_big.tile([128, 512], F32, tag="big")
            for j in range(8):
                w = 8 * g + j
                nc.tensor.matmul(
                    l3_ps[:, 64 * j:64 * (j + 1)],
                    kt[:, 128 * w:128 * (w + 1)],
                    qlmT2[:],
                )
            nc.scalar.activation(e3t[:, 512 * g:512 * (g + 1)], l3_ps[:], Exp)
        cv_ps = ps_acc.tile([64, 65], F32, tag="acc")
        for t in range(NT):
            nc.tensor.matmul(
                cv_ps[:],
                e3t[:, 64 * t:64 * (t + 1)],
                vn[:, t, :],
                start=(t == 0),
                stop=(t == NT - 1),
            )
        d3i = spool.tile([64, 1], F32, tag="d3i")
        nc.vector.reciprocal(d3i[:], cv_ps[:, 64:65])
        cv_sb = spool.tile([64, 64], MMDT, tag="cv_sb")
        nc.vector.tensor_scalar_mul(cv_sb[:], cv_ps[:, 0:64], d3i[:])

        # ---- M2' = [inv_k2 @ CV | 1] ----
        m2_ps = ps_sm.tile([64, 64], F32, tag="sm")
        nc.tensor.matmul(m2_ps[:], vct[:], cv_sb[:])
        m2a = spool.tile([64, 65], MMDT, tag="m2a")
        nc.scalar.copy(m2a[:, 0:64], m2_ps[:])
        nc.vector.memset(m2a[:, 64:65], 1.0)

        # ---- kernel_1: E1^T = exp(Klm_s2 @ Q^T) [m, S] ----
        # cols [0, 2048) = even s-tiles, [2048, 4096) = odd s-tiles
        e1t = epool.tile([64, S], MMDT, tag="e1t")
        for u in range(8):
            l1_ps = ps_big.tile([64, 512], F32, tag="big")
            nc.tensor.matmul(l1_ps[:], klmT2[:], qt[:, 512 * u:512 * (u + 1)])
            nc.scalar.activation(e1t[:, 512 * u:512 * (u + 1)], l1_ps[:], Exp)

        # ---- X' = E1^T.T @ M2', normalize by last column, store ----
        xsb = opool.tile([128, NT, 64], F32, tag="xsb")
        for g in range(8):
            xp_ps = ps_big.tile([128, 4, 65], F32, tag="big")
            for j in range(4):
                w = 4 * g + j
                nc.tensor.matmul(xp_ps[:, j, :], e1t[:, 128 * w:128 * (w + 1)], m2a[:])
            dgi = spool.tile([128, 4], F32, tag="dgi")
            nc.vector.reciprocal(dgi[:], xp_ps[:, :, 64])
            nc.vector.tensor_tensor(
                xsb[:, 4 * g:4 * (g + 1), :],
                xp_ps[:, :, 0:64],
                dgi[:].rearrange("p (a b) -> p a b", b=1).broadcast_to([128, 4, 64]),
                op=Alu.mult,
            )
        nc.sync.dma_start(
            out=x_d[p].rearrange("(t p) d -> p t d", p=128), in_=xsb[:]
        )


def build_nc(n_pairs=PAIRS, reps=1):
    from contextlib import ExitStack

    nc = bacc.Bacc("TRN2", target_bir_lowering=False, debug=False)
    q_d = nc.declare_dram_parameter("Q", [n_pairs, S, D], F32, isOutput=False)
    k_d = nc.declare_dram_parameter("K", [n_pairs, S, D], F32, isOutput=False)
    v_d = nc.declare_dram_parameter("V", [n_pairs, S, D], F32, isOutput=False)
    x_d = nc.declare_dram_parameter("X", [n_pairs, S, D], F32, isOutput=True)
    with tile.TileContext(nc) as tc:
        with ExitStack() as ctx:
            if reps == 1:
                build_body(tc, ctx, q_d[:], k_d[:], v_d[:], x_d[:], n_pairs)
            else:
                with tc.For_i(0, reps, 1):
                    build_body(tc, ctx, q_d[:], k_d[:], v_d[:], x_d[:], n_pairs)
    nc.finalize()
    return nc


_CACHED = {}


def kernel(Q: np.ndarray, K: np.ndarray, V: np.ndarray) -> np.ndarray:
    from concourse.bass_utils import run_bass_kernel_spmd

    if "nc" not in _CACHED:
        _CACHED["nc"] = build_nc()
    nc = _CACHED["nc"]

    qf = np.ascontiguousarray(Q.reshape(B * H, S, D), dtype=np.float32)
    kf = np.ascontiguousarray(K.reshape(B * H, S, D), dtype=np.float32)
    vf = np.ascontiguousarray(V.reshape(B * H, S, D), dtype=np.float32)
    core_ids = list(range(N_CORES))
    in_maps = [
        {
            "Q": qf[c * PAIRS:(c + 1) * PAIRS],
            "K": kf[c * PAIRS:(c + 1) * PAIRS],
            "V": vf[c * PAIRS:(c + 1) * PAIRS],
        }
        for c in core_ids
    ]
    res = run_bass_kernel_spmd(nc, in_maps, core_ids)
    out = np.concatenate([res.results[c]["X"] for c in core_ids], axis=0)
    return out.reshape(B, H, S, D)

